# revision 17
# baseline (speedup 1.0000x reference)
"""BinDevianceLoss on 8 Trainium2 NeuronCores.

Strategy (data-parallel over rows, per sharding hint):
  - Host L2-normalizes X (needed anyway for the positive-pair terms it owns),
    and ships a column-ROTATED normalized X^T to each core so that every core
    runs the identical program: core c's own 1024-row slab always sits at
    columns [0, 1024) of its local operand.
  - Each core computes its [1024, 8192] similarity slab on the PE (bf16,
    fp32 accumulate) and reduces it on the fly (never materializing sim in
    DRAM): per row it returns n_neg = #(sim > min_pos - 0.05) and
    S1 = sum over valid negatives of exp(alpha*(sim - margin)).
    exp(z) ~= log1p(exp(z)) here: the neg-side loss term is ~1e-5 of the
    total loss, so the softplus tail correction is far below tolerance.
  - Same-class entries (incl. diagonal) are excluded on-device by an additive
    -2.5 mask on the 128x128 window at slab-local columns [m*128, (m+1)*128),
    which drives exp() to ~e^-50 ~ 0.
  - Host computes everything precision-critical exactly from O(N*D) data:
    positive-pair terms (4x4 block grams), base (Cauchy-Schwarz bounds the
    global sim max by the diagonal), neg_d (row sums via x_i . sum_j x_j),
    and the final scalar assembly in float64. Any row where the device
    approximations could matter (n_neg == 0 fallback, huge threshold) is
    recomputed exactly on host; with setup_inputs() data this never triggers.
"""

import os
import sys

for _p in ("/opt/trn_rl_repo", "/root/.axon_site/_ro/trn_rl_repo"):
    if os.path.isdir(_p) and _p not in sys.path:
        sys.path.insert(0, _p)

import numpy as np

N = 8192
D = 128
K = 4
ALPHA = 20.0
MARGIN = 0.5
NCORES = 8
SLAB = N // NCORES          # 1024 rows per core
CHUNKS = SLAB // 128        # 8 row chunks of 128
SUPER = 2048                # column supertile (4 PSUM banks)
NSUPER = N // SUPER         # 4
MASK_ADD = -2.5             # additive mask: exp arg lands in [-80, -40]

_NC = None  # compiled program cache


def _build_nc():
    from concourse import bacc, tile, mybir

    nc = bacc.Bacc("TRN2", target_bir_lowering=False, debug=False,
                   num_devices=NCORES)
    bf16 = mybir.dt.bfloat16
    f32 = mybir.dt.float32

    xt_d = nc.dram_tensor("xt", [128, N], bf16, kind="ExternalInput").ap()
    ut_d = nc.dram_tensor("ut", [128, CHUNKS], f32, kind="ExternalInput").ap()
    dmask_d = nc.dram_tensor("dmask", [128, 128], f32, kind="ExternalInput").ap()
    # stats columns: [0:PJ) count = sum(u > ut), [PJ:2PJ) smax = sum(max(u, ut))
    stats_d = nc.dram_tensor("stats", [128, 2 * CHUNKS * NSUPER], f32,
                             kind="ExternalOutput").ap()

    Alu = mybir.AluOpType
    Act = mybir.ActivationFunctionType

    with tile.TileContext(nc) as tc:
        with (
            tc.tile_pool(name="big", bufs=1) as big,
            tc.tile_pool(name="u", bufs=3) as upool,
            tc.tile_pool(name="jk", bufs=2) as jkpool,
            tc.tile_pool(name="ps", bufs=2, space="PSUM") as pspool,
        ):
            # small consts first, on the SWDGE queue so they don't sit
            # behind the 2 MiB xt transfer
            ut = big.tile([128, CHUNKS], f32, tag="ut")
            nc.gpsimd.dma_start(ut[:], ut_d[:])
            dmask = big.tile([128, 128], f32, tag="dmask")
            nc.gpsimd.dma_start(dmask[:], dmask_d[:])
            xt = big.tile([128, N], bf16, tag="xt")
            for j in range(2 * NSUPER):
                h = SUPER // 2
                nc.sync.dma_start(xt[:, j * h:(j + 1) * h],
                                  xt_d[:, j * h:(j + 1) * h])
            bias = big.tile([128, 1], f32, tag="bias")
            nc.gpsimd.memset(bias[:], -float(ALPHA * MARGIN))
            stats = big.tile([128, 2 * CHUNKS * NSUPER], f32, tag="stats")
            PJ = CHUNKS * NSUPER

            for j in range(NSUPER):
                for m in range(CHUNKS):
                    ps = pspool.tile([128, SUPER], f32, tag="ps")
                    for k in range(SUPER // 512):
                        c0 = j * SUPER + k * 512
                        nc.tensor.matmul(
                            ps[:, k * 512:(k + 1) * 512],
                            xt[:, m * 128:(m + 1) * 128],
                            xt[:, c0:c0 + 512],
                            start=True, stop=True,
                        )
                    if j == 0:
                        # same-class window for chunk m: slab-local columns
                        # [m*128, (m+1)*128) -- all inside supertile 0
                        w = ps[:, m * 128:(m + 1) * 128]
                        nc.vector.tensor_tensor(w, w, dmask[:], op=Alu.add)
                    u = upool.tile([128, SUPER], bf16, tag="u")
                    idx = m * NSUPER + j
                    nc.scalar.activation(u[:], ps[:], Act.Exp,
                                         bias=bias[:, 0:1], scale=float(ALPHA))
                    # with accum_out, op1 is the REDUCE op for the accumulator
                    jk1 = jkpool.tile([128, SUPER], bf16, tag="jk1")
                    nc.vector.tensor_scalar(
                        jk1[:], u[:], ut[:, m:m + 1], None, Alu.is_gt, Alu.add,
                        accum_out=stats[:, idx:idx + 1])
                    jk2 = jkpool.tile([128, SUPER], bf16, tag="jk2")
                    nc.vector.tensor_scalar(
                        jk2[:], u[:], ut[:, m:m + 1], None, Alu.max, Alu.add,
                        accum_out=stats[:, PJ + idx:PJ + idx + 1])
            nc.sync.dma_start(stats_d[:], stats[:])
    nc.compile()
    return nc


def _get_nc():
    global _NC
    if _NC is None:
        _NC = _build_nc()
    return _NC


def _softplus64(z):
    return np.log1p(np.exp(-np.abs(z))) + np.maximum(z, 0.0)


def _full_numpy_reference(x, tg):
    """Exact replica of reference.py in numpy (fp32 sims, fp64 assembly).
    Used as a fallback when input structure assumptions fail, and for
    single-row rescues."""
    n = x.shape[0]
    k = K
    xn = x / np.linalg.norm(x, axis=1, keepdims=True)
    same = tg[:, None] == tg[None, :]
    eye = np.eye(n, dtype=bool)
    pos_mask = same & ~eye
    neg_mask = ~same

    BIG = np.float32(1e9)
    pos_sorted = np.empty((n, k - 1), np.float64)
    neg_sorted = np.empty((n, n - k), np.float64)
    gmax = -np.inf
    bs = 512
    for i0 in range(0, n, bs):
        sim = xn[i0:i0 + bs] @ xn.T  # fp32
        gmax = max(gmax, float(sim.max()))
        ps = np.sort(np.where(pos_mask[i0:i0 + bs], sim, BIG), axis=1)[:, :k - 1]
        ns = np.sort(np.where(neg_mask[i0:i0 + bs], sim, BIG), axis=1)[:, :n - k]
        pos_sorted[i0:i0 + bs] = ps
        neg_sorted[i0:i0 + bs] = ns

    base = max(gmax - 0.1, MARGIN + 0.2)
    min_pos = pos_sorted[:, 0]
    neg_valid = neg_sorted > (min_pos - 0.05)[:, None]
    n_neg = neg_valid.sum(axis=1)
    f_neg = _softplus64(ALPHA * (neg_sorted - MARGIN))
    neg_mean = np.where(neg_valid, f_neg, 0.0).sum(axis=1) / np.maximum(n_neg, 1)
    neg_fallback = _softplus64(ALPHA * (neg_sorted[:, -1] - MARGIN))
    neg_loss = (2.0 / ALPHA) * np.where(n_neg > 0, neg_mean, neg_fallback)

    pos_valid = pos_sorted < base
    n_pos = pos_valid.sum(axis=1)
    f_pos = _softplus64(-2.0 * (pos_sorted - MARGIN))
    pos_mean = np.where(pos_valid, f_pos, 0.0).sum(axis=1) / np.maximum(n_pos, 1)
    pos_fallback = _softplus64(-2.0 * (min_pos - MARGIN))
    pos_loss = np.where(n_pos > 0, pos_mean, pos_fallback)

    loss = np.mean(pos_loss + neg_loss)
    prec = np.mean((n_neg == 0).astype(np.float64))
    pos_d = np.mean(pos_sorted)
    neg_d = np.mean(neg_sorted)
    return (np.float32(loss), np.float32(prec), np.float32(pos_d),
            np.float32(neg_d))


def _rescue_row(xn, tg, i):
    """Exact neg-side quantities for one row (fp32 sims, fp64 assembly)."""
    sim = xn @ xn[i]  # [N] fp32
    negm = tg != tg[i]
    negs = sim[negm].astype(np.float64)
    pos_idx = np.where((tg == tg[i]) & (np.arange(len(tg)) != i))[0]
    min_pos = float(sim[pos_idx].min())
    valid = negs > (min_pos - 0.05)
    n_neg = int(valid.sum())
    f = _softplus64(ALPHA * (negs - MARGIN))
    if n_neg > 0:
        neg_term = f[valid].sum() / n_neg
    else:
        neg_term = _softplus64(ALPHA * (negs.max() - MARGIN))
    return n_neg, neg_term


def _run_device(in_maps, trace=False, trace_kwargs=None):
    from concourse import bass_utils
    nc = _get_nc()
    return bass_utils.run_bass_kernel_spmd(
        nc, in_maps, core_ids=list(range(NCORES)), trace=trace,
        **(trace_kwargs or {}))


def _prepare(inputs, targets):
    from concourse import mybir
    bf16_np = mybir.dt.np(mybir.dt.bfloat16)

    x = np.asarray(inputs, dtype=np.float32)
    tg = np.asarray(targets).astype(np.int64)

    norms = np.sqrt((x * x).sum(axis=1, dtype=np.float32))
    xn = (x / norms[:, None]).astype(np.float32)

    # positives from 4x4 block grams (fp32, like the reference's fp32 matmul)
    B = xn.reshape(N // K, K, D)
    G = np.einsum("bik,bjk->bij", B, B).astype(np.float32)  # [2048,4,4]
    ar = np.arange(K)
    diag = G[:, ar, ar].reshape(-1)  # [N] self-sims
    pos = np.stack([G[:, i, [jj for jj in range(K) if jj != i]]
                    for i in range(K)], axis=1)  # [2048, 4, 3]
    pos = pos.reshape(N, K - 1).astype(np.float64)  # positives per row
    pos_sorted = np.sort(pos, axis=1)
    min_pos = pos_sorted[:, 0]
    thresh = min_pos - 0.05
    ut_rows = np.exp(ALPHA * thresh - ALPHA * MARGIN).astype(np.float32)

    xt = np.ascontiguousarray(xn.T).astype(bf16_np)  # [128, 8192]

    same_cls = (tg[:128, None] == tg[None, :128])
    dmask = np.where(same_cls, np.float32(MASK_ADD), np.float32(0.0))
    dmask = np.ascontiguousarray(dmask.astype(np.float32))

    in_maps = []
    for c in range(NCORES):
        s = c * SLAB
        xtc = np.concatenate([xt[:, s:], xt[:, :s]], axis=1)
        utc = np.ascontiguousarray(
            ut_rows[s:s + SLAB].reshape(CHUNKS, 128).T.astype(np.float32))
        in_maps.append({"xt": xtc, "ut": utc, "dmask": dmask})

    host = dict(x=x, tg=tg, xn=xn, G=G, diag=diag, pos_sorted=pos_sorted,
                min_pos=min_pos, thresh=thresh)
    return in_maps, host


def _structure_ok(tg):
    if tg.shape[0] != N:
        return False
    blocks = tg.reshape(N // K, K)
    if not (blocks == blocks[:, :1]).all():
        return False
    if len(np.unique(blocks[:, 0])) != N // K:
        return False
    return True


def _assemble(host, counts, s1):
    """counts, s1: [N] float64 device results. Returns the output tuple."""
    tg = host["tg"]
    xn = host["xn"]
    G = host["G"].astype(np.float64)
    diag = host["diag"].astype(np.float64)
    pos_sorted = host["pos_sorted"]
    min_pos = host["min_pos"]
    thresh = host["thresh"]

    n_neg = np.rint(counts).astype(np.int64)

    # base: |sim| <= max_i ||xn_i||^2 + eps (Cauchy-Schwarz); diagonal is ~1
    nrm2 = diag  # fp32 self-dots of normalized rows
    gmax_lo = float(max(nrm2.max(), pos_sorted.max()))
    gmax_hi = float(nrm2.max()) + 1e-6
    base_lo = max(gmax_lo - 0.1, MARGIN + 0.2)
    base_hi = max(gmax_hi - 0.1, MARGIN + 0.2)
    if np.any((pos_sorted > base_lo - 1e-6) & (pos_sorted < base_hi + 1e-6)):
        # a positive is too close to base to resolve without the full sim max
        return _full_numpy_reference(host["x"], tg)
    base = base_lo

    # pos side (exact, fp64)
    pos_valid = pos_sorted < base
    n_pos = pos_valid.sum(axis=1)
    f_pos = _softplus64(-2.0 * (pos_sorted - MARGIN))
    pos_mean = np.where(pos_valid, f_pos, 0.0).sum(axis=1) / np.maximum(n_pos, 1)
    pos_fallback = _softplus64(-2.0 * (min_pos - MARGIN))
    pos_loss = np.where(n_pos > 0, pos_mean, pos_fallback)

    # neg side from device stats
    neg_term = s1 / np.maximum(n_neg, 1)

    # rescue rows where the fast path can't be trusted: n_neg near 0 (a bf16
    # boundary flip could change the fallback branch) or an unusually high
    # threshold (where the unmasked-tail bound weakens)
    rescue = (n_neg <= 3) | (thresh > 0.2)
    for i in np.nonzero(rescue)[0]:
        nn, nt = _rescue_row(xn, tg, int(i))
        n_neg[i] = nn
        neg_term[i] = nt
    neg_loss = (2.0 / ALPHA) * neg_term

    loss = float(np.mean(pos_loss + neg_loss))
    prec = float(np.mean(n_neg == 0))
    pos_d = float(np.mean(pos_sorted))

    # neg_d: sum over all sims minus same-class part, via row sums
    g = xn.astype(np.float64).sum(axis=0)
    rowsum = xn.astype(np.float64) @ g
    same_sum = G.sum(axis=2).reshape(-1)  # per-row same-class incl self
    neg_d = float((rowsum - same_sum).sum() / (N * (N - K)))

    return (np.float32(loss), np.float32(prec), np.float32(pos_d),
            np.float32(neg_d))


def _kernel_impl(inputs, targets, trace=False, trace_kwargs=None):
    tg = np.asarray(targets).astype(np.int64)
    x = np.asarray(inputs, dtype=np.float32)
    if not _structure_ok(tg):
        return _full_numpy_reference(x, tg), None

    in_maps, host = _prepare(x, tg)
    res = _run_device(in_maps, trace=trace, trace_kwargs=trace_kwargs)

    counts = np.empty(N, np.float64)
    smax = np.empty(N, np.float64)
    PJ = CHUNKS * NSUPER
    for c in range(NCORES):
        st = res.results[c]["stats"].astype(np.float64)  # [128, 2*PJ]
        s = c * SLAB
        # row (s + m*128 + p) lives at [p, m]; sum the NSUPER partials
        for arr, lo in ((counts, 0), (smax, PJ)):
            parts = st[:, lo:lo + PJ].reshape(128, CHUNKS, NSUPER).sum(axis=2)
            arr[s:s + SLAB] = parts.T.reshape(-1)

    # sum over valid negatives of u:
    #   sum(max(u, ut)) = S1 + ut*(Ncols - count)  =>  S1 = smax - ut*(N - count)
    ut64 = np.exp(ALPHA * host["thresh"] - ALPHA * MARGIN)
    s1 = np.maximum(smax - ut64 * (N - counts), 0.0)
    return _assemble(host, counts, s1), res


def kernel(inputs, targets):
    out, _ = _kernel_impl(inputs, targets)
    return out


# revision 46
# speedup vs baseline: 1.0185x; 1.0185x over previous
"""BinDevianceLoss on 8 Trainium2 NeuronCores.

Strategy (data-parallel over rows, per sharding hint):
  - Host L2-normalizes X (needed anyway for the positive-pair terms it owns),
    and ships a column-ROTATED normalized X^T to each core so that every core
    runs the identical program: core c's own 1024-row slab always sits at
    columns [0, 1024) of its local operand.
  - Each core computes its [1024, 8192] similarity slab on the PE (bf16,
    fp32 accumulate) and reduces it on the fly (never materializing sim in
    DRAM): per row it returns n_neg = #(sim > min_pos - 0.05) and
    S1 = sum over valid negatives of exp(alpha*(sim - margin)).
    exp(z) ~= log1p(exp(z)) here: the neg-side loss term is ~1e-5 of the
    total loss, so the softplus tail correction is far below tolerance.
  - Same-class entries (incl. diagonal) are excluded on-device by an additive
    -2.5 mask on the 128x128 window at slab-local columns [m*128, (m+1)*128),
    which drives exp() to ~e^-50 ~ 0.
  - Host computes everything precision-critical exactly from O(N*D) data:
    positive-pair terms (4x4 block grams), base (Cauchy-Schwarz bounds the
    global sim max by the diagonal), neg_d (row sums via x_i . sum_j x_j),
    and the final scalar assembly in float64. Any row where the device
    approximations could matter (n_neg == 0 fallback, huge threshold) is
    recomputed exactly on host; with setup_inputs() data this never triggers.
"""

import os
import sys

for _p in ("/opt/trn_rl_repo", "/root/.axon_site/_ro/trn_rl_repo"):
    if os.path.isdir(_p) and _p not in sys.path:
        sys.path.insert(0, _p)

import numpy as np

N = 8192
D = 128
K = 4
ALPHA = 20.0
MARGIN = 0.5
NCORES = 8
SLAB = N // NCORES          # 1024 rows per core
CHUNKS = SLAB // 128        # 8 row chunks of 128
SUPER = 2048                # column supertile (4 PSUM banks)
NSUPER = N // SUPER         # 4
MASK_ADD = -2.5             # additive mask: exp arg lands in [-80, -40]

_NC = None  # compiled program cache


def _build_nc():
    from concourse import bacc, tile, mybir

    nc = bacc.Bacc("TRN2", target_bir_lowering=False, debug=False,
                   num_devices=NCORES)
    bf16 = mybir.dt.bfloat16
    f32 = mybir.dt.float32

    xt_d = nc.dram_tensor("xt", [128, N], bf16, kind="ExternalInput").ap()
    ut_d = nc.dram_tensor("ut", [128, CHUNKS], f32, kind="ExternalInput").ap()
    # the block-diagonal same-class mask is rank-32: mask = em^T @ fm with
    # em[b,p] = [p//4==b], fm[b,c] = MASK_ADD*[c//4==b] -- applied as one
    # accumulating K=32 matmul on the idle PE instead of a DVE pass
    em_d = nc.dram_tensor("em", [32, 128], bf16, kind="ExternalInput").ap()
    fm_d = nc.dram_tensor("fm", [32, 128], bf16, kind="ExternalInput").ap()
    # stats columns: [0:PJ) count = sum(u > ut), [PJ:2PJ) smax = sum(max(u, ut))
    stats_d = nc.dram_tensor("stats", [128, 2 * CHUNKS * NSUPER], f32,
                             kind="ExternalOutput").ap()

    Alu = mybir.AluOpType
    Act = mybir.ActivationFunctionType

    # Supertiles offloaded from the saturated ACT engine to the idle DVE via
    # the Schraudolph bit-trick: e^(20s-10) ~= bitcast_f32(int32(A*s + B)),
    # one fused tensor_scalar (fp32 ALU, int32 output conversion). +-3% ripple
    # on these quarters only; validated end-to-end at loss rel err 9e-8.
    HYBRID = {(1, 1), (3, 2), (5, 3), (7, 1)}
    LN2 = 0.6931471805599453
    SCHRA_A = float(ALPHA * 8388608.0 / LN2)
    SCHRA_B = float(-ALPHA * MARGIN * 8388608.0 / LN2 + 127.0 * 8388608.0
                    - 350000.0)

    with tile.TileContext(nc) as tc:
        with (
            tc.tile_pool(name="big", bufs=1) as big,
            tc.tile_pool(name="u", bufs=4) as upool,
            tc.tile_pool(name="jk", bufs=2) as jkpool,
            tc.tile_pool(name="ps", bufs=2, space="PSUM") as pspool,
        ):
            # small consts on the SWDGE queue: they transfer in parallel with
            # the 2 MiB xt stream on the HWDGE queue (both land before use)
            em = big.tile([32, 128], bf16, tag="em")
            nc.gpsimd.dma_start(em[:], em_d[:])
            fm = big.tile([32, 128], bf16, tag="fm")
            nc.gpsimd.dma_start(fm[:], fm_d[:])
            ut = big.tile([128, CHUNKS], f32, tag="ut")
            nc.gpsimd.dma_start(ut[:], ut_d[:])
            xt = big.tile([128, N], bf16, tag="xt")
            for j in range(2 * NSUPER):
                h = SUPER // 2
                nc.sync.dma_start(xt[:, j * h:(j + 1) * h],
                                  xt_d[:, j * h:(j + 1) * h])
            bias = big.tile([128, 1], f32, tag="bias")
            nc.gpsimd.memset(bias[:], -float(ALPHA * MARGIN))
            stats = big.tile([128, 2 * CHUNKS * NSUPER], f32, tag="stats")
            PJ = CHUNKS * NSUPER

            for j in range(NSUPER):
                for m in range(CHUNKS):
                    ps = pspool.tile([128, SUPER], f32, tag="ps")
                    for k in range(SUPER // 512):
                        c0 = j * SUPER + k * 512
                        nc.tensor.matmul(
                            ps[:, k * 512:(k + 1) * 512],
                            xt[:, m * 128:(m + 1) * 128],
                            xt[:, c0:c0 + 512],
                            start=True, stop=True,
                        )
                    if j == 0:
                        # same-class window for chunk m: slab-local columns
                        # [m*128, (m+1)*128) -- all inside supertile 0.
                        # accumulate the rank-32 mask onto it via the PE
                        nc.tensor.matmul(ps[:, m * 128:(m + 1) * 128],
                                         em[:], fm[:], start=False, stop=True,
                                         skip_group_check=True)
                    idx = m * NSUPER + j
                    if (m, j) in HYBRID:
                        ui = upool.tile([128, SUPER], mybir.dt.int32, tag="ui")
                        nc.vector.tensor_scalar(
                            ui[:], ps[:], SCHRA_A, SCHRA_B, Alu.mult, Alu.add)
                        uv = ui[:].bitcast(mybir.dt.float32)
                    else:
                        u = upool.tile([128, SUPER], bf16, tag="u")
                        nc.scalar.activation(u[:], ps[:], Act.Exp,
                                             bias=bias[:, 0:1],
                                             scale=float(ALPHA))
                        uv = u[:]
                    # with accum_out, op1 is the accumulator's REDUCE op
                    jk1 = jkpool.tile([128, SUPER], bf16, tag="jk1")
                    nc.vector.tensor_scalar(
                        jk1[:], uv, ut[:, m:m + 1], None, Alu.is_gt, Alu.add,
                        accum_out=stats[:, idx:idx + 1])
                    jk2 = jkpool.tile([128, SUPER], bf16, tag="jk2")
                    nc.vector.tensor_scalar(
                        jk2[:], uv, ut[:, m:m + 1], None, Alu.max, Alu.add,
                        accum_out=stats[:, PJ + idx:PJ + idx + 1])
            nc.sync.dma_start(stats_d[:], stats[:])
    nc.compile()
    return nc


def _get_nc():
    global _NC
    if _NC is None:
        _NC = _build_nc()
    return _NC


def _softplus64(z):
    return np.log1p(np.exp(-np.abs(z))) + np.maximum(z, 0.0)


def _full_numpy_reference(x, tg):
    """Exact replica of reference.py in numpy (fp32 sims, fp64 assembly).
    Used as a fallback when input structure assumptions fail, and for
    single-row rescues."""
    n = x.shape[0]
    k = K
    xn = x / np.linalg.norm(x, axis=1, keepdims=True)
    same = tg[:, None] == tg[None, :]
    eye = np.eye(n, dtype=bool)
    pos_mask = same & ~eye
    neg_mask = ~same

    BIG = np.float32(1e9)
    pos_sorted = np.empty((n, k - 1), np.float64)
    neg_sorted = np.empty((n, n - k), np.float64)
    gmax = -np.inf
    bs = 512
    for i0 in range(0, n, bs):
        sim = xn[i0:i0 + bs] @ xn.T  # fp32
        gmax = max(gmax, float(sim.max()))
        ps = np.sort(np.where(pos_mask[i0:i0 + bs], sim, BIG), axis=1)[:, :k - 1]
        ns = np.sort(np.where(neg_mask[i0:i0 + bs], sim, BIG), axis=1)[:, :n - k]
        pos_sorted[i0:i0 + bs] = ps
        neg_sorted[i0:i0 + bs] = ns

    base = max(gmax - 0.1, MARGIN + 0.2)
    min_pos = pos_sorted[:, 0]
    neg_valid = neg_sorted > (min_pos - 0.05)[:, None]
    n_neg = neg_valid.sum(axis=1)
    f_neg = _softplus64(ALPHA * (neg_sorted - MARGIN))
    neg_mean = np.where(neg_valid, f_neg, 0.0).sum(axis=1) / np.maximum(n_neg, 1)
    neg_fallback = _softplus64(ALPHA * (neg_sorted[:, -1] - MARGIN))
    neg_loss = (2.0 / ALPHA) * np.where(n_neg > 0, neg_mean, neg_fallback)

    pos_valid = pos_sorted < base
    n_pos = pos_valid.sum(axis=1)
    f_pos = _softplus64(-2.0 * (pos_sorted - MARGIN))
    pos_mean = np.where(pos_valid, f_pos, 0.0).sum(axis=1) / np.maximum(n_pos, 1)
    pos_fallback = _softplus64(-2.0 * (min_pos - MARGIN))
    pos_loss = np.where(n_pos > 0, pos_mean, pos_fallback)

    loss = np.mean(pos_loss + neg_loss)
    prec = np.mean((n_neg == 0).astype(np.float64))
    pos_d = np.mean(pos_sorted)
    neg_d = np.mean(neg_sorted)
    return (np.float32(loss), np.float32(prec), np.float32(pos_d),
            np.float32(neg_d))


def _rescue_row(xn, tg, i):
    """Exact neg-side quantities for one row (fp32 sims, fp64 assembly)."""
    sim = xn @ xn[i]  # [N] fp32
    negm = tg != tg[i]
    negs = sim[negm].astype(np.float64)
    pos_idx = np.where((tg == tg[i]) & (np.arange(len(tg)) != i))[0]
    min_pos = float(sim[pos_idx].min())
    valid = negs > (min_pos - 0.05)
    n_neg = int(valid.sum())
    f = _softplus64(ALPHA * (negs - MARGIN))
    if n_neg > 0:
        neg_term = f[valid].sum() / n_neg
    else:
        neg_term = _softplus64(ALPHA * (negs.max() - MARGIN))
    return n_neg, neg_term


def _run_device(in_maps, trace=False, trace_kwargs=None):
    from concourse import bass_utils
    nc = _get_nc()
    return bass_utils.run_bass_kernel_spmd(
        nc, in_maps, core_ids=list(range(NCORES)), trace=trace,
        **(trace_kwargs or {}))


def _prepare(inputs, targets):
    from concourse import mybir
    bf16_np = mybir.dt.np(mybir.dt.bfloat16)

    x = np.asarray(inputs, dtype=np.float32)
    tg = np.asarray(targets).astype(np.int64)

    norms = np.sqrt((x * x).sum(axis=1, dtype=np.float32))
    xn = (x / norms[:, None]).astype(np.float32)

    # positives from 4x4 block grams (fp32, like the reference's fp32 matmul)
    B = xn.reshape(N // K, K, D)
    G = np.einsum("bik,bjk->bij", B, B).astype(np.float32)  # [2048,4,4]
    ar = np.arange(K)
    diag = G[:, ar, ar].reshape(-1)  # [N] self-sims
    pos = np.stack([G[:, i, [jj for jj in range(K) if jj != i]]
                    for i in range(K)], axis=1)  # [2048, 4, 3]
    pos = pos.reshape(N, K - 1).astype(np.float64)  # positives per row
    pos_sorted = np.sort(pos, axis=1)
    min_pos = pos_sorted[:, 0]
    thresh = min_pos - 0.05
    ut_rows = np.exp(ALPHA * thresh - ALPHA * MARGIN).astype(np.float32)

    xt = np.ascontiguousarray(xn.T).astype(bf16_np)  # [128, 8192]

    # rank-32 factorization of the block-diagonal mask (classes of K=4
    # within any aligned 128-window): mask = em^T @ fm
    blk = (np.arange(128) // K)
    em = (blk[None, :] == np.arange(32)[:, None]).astype(np.float32)
    fm = (em * np.float32(MASK_ADD)).astype(bf16_np)
    em = em.astype(bf16_np)

    in_maps = []
    for c in range(NCORES):
        s = c * SLAB
        xtc = np.concatenate([xt[:, s:], xt[:, :s]], axis=1)
        utc = np.ascontiguousarray(
            ut_rows[s:s + SLAB].reshape(CHUNKS, 128).T.astype(np.float32))
        in_maps.append({"xt": xtc, "ut": utc, "em": em, "fm": fm})

    host = dict(x=x, tg=tg, xn=xn, G=G, diag=diag, pos_sorted=pos_sorted,
                min_pos=min_pos, thresh=thresh)
    return in_maps, host


def _structure_ok(tg):
    if tg.shape[0] != N:
        return False
    blocks = tg.reshape(N // K, K)
    if not (blocks == blocks[:, :1]).all():
        return False
    if len(np.unique(blocks[:, 0])) != N // K:
        return False
    return True


def _assemble(host, counts, s1):
    """counts, s1: [N] float64 device results. Returns the output tuple."""
    tg = host["tg"]
    xn = host["xn"]
    G = host["G"].astype(np.float64)
    diag = host["diag"].astype(np.float64)
    pos_sorted = host["pos_sorted"]
    min_pos = host["min_pos"]
    thresh = host["thresh"]

    n_neg = np.rint(counts).astype(np.int64)

    # base: |sim| <= max_i ||xn_i||^2 + eps (Cauchy-Schwarz); diagonal is ~1
    nrm2 = diag  # fp32 self-dots of normalized rows
    gmax_lo = float(max(nrm2.max(), pos_sorted.max()))
    gmax_hi = float(nrm2.max()) + 1e-6
    base_lo = max(gmax_lo - 0.1, MARGIN + 0.2)
    base_hi = max(gmax_hi - 0.1, MARGIN + 0.2)
    if np.any((pos_sorted > base_lo - 1e-6) & (pos_sorted < base_hi + 1e-6)):
        # a positive is too close to base to resolve without the full sim max
        return _full_numpy_reference(host["x"], tg)
    base = base_lo

    # pos side (exact, fp64)
    pos_valid = pos_sorted < base
    n_pos = pos_valid.sum(axis=1)
    f_pos = _softplus64(-2.0 * (pos_sorted - MARGIN))
    pos_mean = np.where(pos_valid, f_pos, 0.0).sum(axis=1) / np.maximum(n_pos, 1)
    pos_fallback = _softplus64(-2.0 * (min_pos - MARGIN))
    pos_loss = np.where(n_pos > 0, pos_mean, pos_fallback)

    # neg side from device stats
    neg_term = s1 / np.maximum(n_neg, 1)

    # rescue rows where the fast path can't be trusted: n_neg near 0 (a bf16
    # boundary flip could change the fallback branch) or an unusually high
    # threshold (where the unmasked-tail bound weakens)
    rescue = (n_neg <= 3) | (thresh > 0.2)
    for i in np.nonzero(rescue)[0]:
        nn, nt = _rescue_row(xn, tg, int(i))
        n_neg[i] = nn
        neg_term[i] = nt
    neg_loss = (2.0 / ALPHA) * neg_term

    loss = float(np.mean(pos_loss + neg_loss))
    prec = float(np.mean(n_neg == 0))
    pos_d = float(np.mean(pos_sorted))

    # neg_d: sum over all sims minus same-class part, via row sums
    g = xn.astype(np.float64).sum(axis=0)
    rowsum = xn.astype(np.float64) @ g
    same_sum = G.sum(axis=2).reshape(-1)  # per-row same-class incl self
    neg_d = float((rowsum - same_sum).sum() / (N * (N - K)))

    return (np.float32(loss), np.float32(prec), np.float32(pos_d),
            np.float32(neg_d))


def _kernel_impl(inputs, targets, trace=False, trace_kwargs=None):
    tg = np.asarray(targets).astype(np.int64)
    x = np.asarray(inputs, dtype=np.float32)
    if not _structure_ok(tg):
        return _full_numpy_reference(x, tg), None

    in_maps, host = _prepare(x, tg)
    res = _run_device(in_maps, trace=trace, trace_kwargs=trace_kwargs)

    counts = np.empty(N, np.float64)
    smax = np.empty(N, np.float64)
    PJ = CHUNKS * NSUPER
    for c in range(NCORES):
        st = res.results[c]["stats"].astype(np.float64)  # [128, 2*PJ]
        s = c * SLAB
        # row (s + m*128 + p) lives at [p, m]; sum the NSUPER partials
        for arr, lo in ((counts, 0), (smax, PJ)):
            parts = st[:, lo:lo + PJ].reshape(128, CHUNKS, NSUPER).sum(axis=2)
            arr[s:s + SLAB] = parts.T.reshape(-1)

    # sum over valid negatives of u:
    #   sum(max(u, ut)) = S1 + ut*(Ncols - count)  =>  S1 = smax - ut*(N - count)
    ut64 = np.exp(ALPHA * host["thresh"] - ALPHA * MARGIN)
    s1 = np.maximum(smax - ut64 * (N - counts), 0.0)
    return _assemble(host, counts, s1), res


def kernel(inputs, targets):
    out, _ = _kernel_impl(inputs, targets)
    return out
